# revision 19
# baseline (speedup 1.0000x reference)
"""Single-head attention (B=8, N=2048, D=512, fp32) on 8 TRN2 NeuronCores.

Sharding: data-parallel over batch — core i computes batch element i
end-to-end (weights replicated). Host passes x already transposed
(xT [D, N]) and weights pre-rearranged to [ki, ko, dout], so the kernel
starts projecting immediately — no on-device transposes at all.

Per-core pipeline:
  QT = Wq^T-contract -> [512,2048]   (D on partitions, f32r)
  KT likewise; V = x @ Wv -> [2048,512] (seq on partitions, bf16)
  per 512-wide q strip:
    for each 128-row k tile kt:
      S^T tile [k=128,q=512] = KT-chunk^T @ QT    (f32r, accum over D chunks)
      E = exp(S^T / sqrt(D)) -> bf16              (ACT, 2 half-tiles)
      per 128-col q subtile qt:
        O[qt]    += E[:,qt]^T @ V[kt]             (bf16 in, fp32 PSUM accum)
        dsum[qt] += E[:,qt]^T @ ones              (N=1 bf16 matmul)
    O[qt] *= 1/dsum[qt] (per-partition scalar, DVE/ACT alternating); DMA out

Inputs are declared float32r in DRAM (bit-identical to f32): no on-chip
casts. bf16 E/V keep the O-side LDWEIGHTS on the fast FWL path and make
the N=1 dsum matmuls legal (fp32r moving requires even free sizes).
DMA: xT strips + weights interleaved across both HW DGE queues
(strip-major so strip-s projections start as soon as strip s lands),
biases on the software DGE, outputs alternate sync/scalar.
"""

import numpy as np

import concourse.bass as bass
import concourse.tile as tile
from concourse import bacc, mybir
from concourse import bass_utils
from concourse.bass import ts
from contextlib import ExitStack

B, N, D = 8, 2048, 512
P = 128
NT = N // P      # 16 seq tiles
DC = D // P      # 4 d chunks
QS = 512         # q-strip width (one PSUM bank of fp32)
NS = N // QS     # 4 strips
QT_PER = QS // P # 4 q subtiles per strip
SOFTMAX_SCALE = 1.0 / float(np.sqrt(D))

F32 = mybir.dt.float32
F32R = mybir.dt.float32r
BF16 = mybir.dt.bfloat16
AF = mybir.ActivationFunctionType


def _build():
    nc = bacc.Bacc("TRN2", target_bir_lowering=False, debug=False)

    # xt is x^T [D, N]; weights are pre-rearranged [ki, ko*dout] (host-side)
    xt = nc.dram_tensor("xt", [D, N], F32R, kind="ExternalInput").ap()
    wq = nc.dram_tensor("wq", [P, DC * D], F32R, kind="ExternalInput").ap()
    bq = nc.dram_tensor("bq", [D], F32, kind="ExternalInput").ap()
    wk = nc.dram_tensor("wk", [P, DC * D], F32R, kind="ExternalInput").ap()
    bk = nc.dram_tensor("bk", [D], F32, kind="ExternalInput").ap()
    wv = nc.dram_tensor("wv", [P, DC * D], F32R, kind="ExternalInput").ap()
    bv = nc.dram_tensor("bv", [D], F32, kind="ExternalInput").ap()
    out = nc.dram_tensor("out", [N, D], BF16, kind="ExternalOutput").ap()

    with ExitStack() as ctx:
        tc = ctx.enter_context(tile.TileContext(nc))

        const = ctx.enter_context(tc.tile_pool(name="const", bufs=1))
        wpool = ctx.enter_context(tc.tile_pool(name="wpool", bufs=3))
        big = ctx.enter_context(tc.tile_pool(name="big", bufs=1))
        epool = ctx.enter_context(tc.tile_pool(name="epool", bufs=4))
        opool = ctx.enter_context(tc.tile_pool(name="opool", bufs=3))
        rpool = ctx.enter_context(tc.tile_pool(name="rpool", bufs=2))

        ones_col = const.tile([P, 1], BF16)
        nc.vector.memset(ones_col, 1.0)

        # biases via software DGE (tiny; keep HW queues free for x/weights)
        bq_sb = const.tile([P, DC], F32)
        nc.gpsimd.dma_start(bq_sb[:], bq.rearrange("(c p) -> p c", p=P))
        bk_sb = const.tile([P, DC], F32)
        nc.gpsimd.dma_start(bk_sb[:], bk.rearrange("(c p) -> p c", p=P))
        bv_rep = const.tile([P, D], F32)
        nc.gpsimd.dma_start(bv_rep[:], bv[None, :].to_broadcast((P, D)))

        # big persistent tensors
        xT = big.tile([P, DC, N], F32R)    # x^T: d on partitions
        QT = big.tile([P, DC, N], F32R)
        KT = big.tile([P, DC, N], F32R)
        V = big.tile([P, NT * D], BF16)    # natural: seq on partitions (bf16)

        # weights: wq/wk in co-major [ki, co, ko, 128] layout, loaded one
        # co-chunk (256KB) at a time interleaved with the xT strips so the
        # first projection matmul only waits for ~512KB of DMA; wv (needed
        # a strip later) rides along after strip 1
        wq_sb = wpool.tile([P, DC, DC, P], F32R, tag="wq")
        wk_sb = wpool.tile([P, DC, DC, P], F32R, tag="wk")
        wv_sb = wpool.tile([P, DC, D], F32R, tag="wv")
        w_sb = {"q": wq_sb, "k": wk_sb, "v": wv_sb}
        wq4 = wq.rearrange("p (co k) -> p co k", co=DC)
        wk4 = wk.rearrange("p (co k) -> p co k", co=DC)
        nc.scalar.dma_start(w_sb["q"][:, 0], wq4[:, 0])
        nc.sync.dma_start(w_sb["k"][:, 0], wk4[:, 0])

        # xT strips with one wq/wk co-chunk trailing each strip: projections
        # run co-major over strips, so co-chunk c is only needed after
        # ~(c+1) strips' worth of PE work — DMA stays ahead throughout
        for s in range(NS):
            for c in range(DC):
                eng = nc.sync if (c % 2 == 0) else nc.scalar
                eng.dma_start(xT[:, c, ts(s, QS)], xt[ts(c, P), ts(s, QS)])
            if s < DC - 1:
                nc.scalar.dma_start(w_sb["q"][:, s + 1], wq4[:, s + 1])
                nc.sync.dma_start(w_sb["k"][:, s + 1], wk4[:, s + 1])
            if s == 2:
                nc.sync.dma_start(w_sb["v"][:], wv)

        # ---- phase 1: projections, strip-major ----
        with tc.tile_pool(name="ps_proj", bufs=4, space="PSUM") as ps_proj:
            for co in range(DC):
                for s in range(NS):
                    for name, dst, b_sb in (("q", QT, bq_sb), ("k", KT, bk_sb)):
                        wr = w_sb[name]
                        pq = ps_proj.tile([P, QS], F32, tag="proj")
                        for ki in range(DC):
                            nc.tensor.matmul(
                                pq[:], wr[:, co, ki, :], xT[:, ki, ts(s, QS)],
                                start=(ki == 0), stop=(ki == DC - 1),
                            )
                        # bias add (per-partition) + round to fp32r on ACT
                        nc.scalar.activation(
                            dst[:, co, ts(s, QS)], pq[:], AF.Identity,
                            bias=b_sb[:, co:co + 1],
                        )
            # V last (phase 2 needs all of it anyway, and this lets wv's
            # DMA trail the wq/wk chunks): natural layout, bias along the
            # free dim via replicated tile
            wr = w_sb["v"]
            for m in range(NT):
                pv = ps_proj.tile([P, QS], F32, tag="proj")
                for ki in range(DC):
                    nc.tensor.matmul(
                        pv[:], xT[:, ki, ts(m, P)], wr[:, ki, :],
                        start=(ki == 0), stop=(ki == DC - 1),
                    )
                nc.vector.tensor_add(out=V[:, ts(m, D)], in0=pv[:],
                                     in1=bv_rep[:])

        # ---- phase 2: attention, natural-layout O accumulation ----
        with tc.tile_pool(name="ps_st", bufs=3, space="PSUM") as ps_st, \
             tc.tile_pool(name="ps_o", bufs=4, space="PSUM") as ps_o, \
             tc.tile_pool(name="ps_ds", bufs=1, space="PSUM") as ps_ds:
            for s in range(NS):
                o_ps = [ps_o.tile([P, QS], F32, tag="o", name=f"o_{s}_{qt}")
                        for qt in range(QT_PER)]
                dsum = ps_ds.tile([P, QT_PER], F32, tag="ds")
                # zero data; all dsum matmuls accumulate with start=False so
                # correctness doesn't depend on has_written clear granularity
                nc.vector.memset(dsum, 0.0)

                def s_part(kt, cs, st):
                    for c in cs:
                        nc.tensor.matmul(
                            st[:], KT[:, c, ts(kt, P)], QT[:, c, ts(s, QS)],
                            start=(c == 0), stop=(c == DC - 1),
                        )

                def exp_of(st):
                    e = epool.tile([P, QS], BF16, tag="e")
                    nc.scalar.activation(e[:], st[:], AF.Exp,
                                         scale=SOFTMAX_SCALE)
                    return e

                def o_pair(qt, e_cur, kt):
                    nc.tensor.matmul(
                        o_ps[qt][:], e_cur[:, ts(qt, P)], V[:, ts(kt, D)],
                        start=(kt == 0), stop=(kt == NT - 1),
                        skip_group_check=True,
                    )
                    nc.tensor.matmul(
                        dsum[:, qt:qt + 1], e_cur[:, ts(qt, P)],
                        ones_col[:],
                        start=False, stop=(kt == NT - 1),
                        skip_group_check=True,
                    )

                # one-iteration software pipeline, with the S matmuls of
                # kt+1 woven between O pairs of kt so the f32r LDWEIGHTS
                # bursts never queue up in front of an O matmul
                st0 = ps_st.tile([P, QS], F32, tag="st", name="st_p")
                s_part(0, range(DC), st0)
                e_cur = exp_of(st0)
                for kt in range(NT):
                    if kt + 1 < NT:
                        st = ps_st.tile([P, QS], F32, tag="st")
                        s_part(kt + 1, (0, 1), st)
                        o_pair(0, e_cur, kt)
                        o_pair(1, e_cur, kt)
                        s_part(kt + 1, (2, 3), st)
                        e_nxt = exp_of(st)
                        o_pair(2, e_cur, kt)
                        o_pair(3, e_cur, kt)
                    else:
                        for qt in range(QT_PER):
                            o_pair(qt, e_cur, kt)
                        e_nxt = None
                    e_cur = e_nxt
                r = rpool.tile([P, QT_PER], F32, tag="r")
                for qt in range(QT_PER):
                    # per-qt recip so the strip tail pipelines per tile
                    nc.vector.reciprocal(r[:, qt:qt + 1], dsum[:, qt:qt + 1])
                    ob = opool.tile([P, QS], BF16, tag="ob")
                    if qt % 2 == 0:
                        nc.vector.tensor_scalar_mul(ob[:], o_ps[qt][:],
                                                    r[:, qt:qt + 1])
                    else:
                        nc.scalar.activation(ob[:], o_ps[qt][:], AF.Identity,
                                             scale=r[:, qt:qt + 1])
                    eng = nc.sync if (qt % 2 == 0) else nc.scalar
                    eng.dma_start(out[ts(s * QT_PER + qt, P), :], ob[:])

    nc.compile()
    return nc


_CACHE = {}


def _get_nc():
    if "nc" not in _CACHE:
        _CACHE["nc"] = _build()
    return _CACHE["nc"]


def _host_prep_w_qk(w):
    # [din, dout] -> [ki, co, ko, 128] flattened, contiguous: one 2KB run
    # per partition per co so per-co weight DMAs use full descriptors
    w = np.asarray(w, dtype=np.float32)
    return np.ascontiguousarray(
        w.reshape(DC, P, DC, P).transpose(1, 2, 0, 3).reshape(P, DC * D))


def _host_prep_w_v(w):
    # [din, dout] -> [ki, ko, dout] flattened to [128, DC*D], contiguous
    w = np.asarray(w, dtype=np.float32)
    return np.ascontiguousarray(
        w.reshape(DC, P, D).transpose(1, 0, 2).reshape(P, DC * D))


def kernel(x, Wq_w, Wq_b, Wk_w, Wk_b, Wv_w, Wv_b, _trace=False, _tmpdir=None):
    nc = _get_nc()
    x = np.asarray(x, dtype=np.float32)
    xt = np.ascontiguousarray(x.transpose(0, 2, 1))   # [B, D, N]
    args = {
        "wq": _host_prep_w_qk(Wq_w),
        "bq": np.ascontiguousarray(Wq_b, np.float32),
        "wk": _host_prep_w_qk(Wk_w),
        "bk": np.ascontiguousarray(Wk_b, np.float32),
        "wv": _host_prep_w_v(Wv_w),
        "bv": np.ascontiguousarray(Wv_b, np.float32),
    }
    in_maps = [dict(args, xt=xt[i]) for i in range(B)]
    res = bass_utils.run_bass_kernel_spmd(
        nc, in_maps, core_ids=list(range(B)),
        trace=_trace, tmpdir=_tmpdir,
    )
    out = np.stack([np.asarray(r["out"], dtype=np.float32)
                    for r in res.results], axis=0)
    if _trace:
        kernel.last_results = res
    return out


if __name__ == "__main__":
    rng = np.random.default_rng(0)
    inputs = {
        "x": rng.standard_normal((B, N, D)).astype(np.float32),
        "Wq_w": (0.02 * rng.standard_normal((D, D))).astype(np.float32),
        "Wq_b": np.zeros(D, np.float32),
        "Wk_w": (0.02 * rng.standard_normal((D, D))).astype(np.float32),
        "Wk_b": np.zeros(D, np.float32),
        "Wv_w": (0.02 * rng.standard_normal((D, D))).astype(np.float32),
        "Wv_b": np.zeros(D, np.float32),
    }
    got = kernel(**inputs)
    print("out shape:", got.shape, got.dtype)


# revision 20
# speedup vs baseline: 1.1378x; 1.1378x over previous
"""Single-head attention (B=8, N=2048, D=512, fp32) on 8 TRN2 NeuronCores.

Sharding: data-parallel over batch — core i computes batch element i
end-to-end (weights replicated). Host passes x already transposed
(xT [D, N]) and weights pre-rearranged to [ki, ko, dout], so the kernel
starts projecting immediately — no on-device transposes at all.

Per-core pipeline:
  QT = Wq^T-contract -> [512,2048]   (D on partitions, f32r)
  KT likewise; V = x @ Wv -> [2048,512] (seq on partitions, bf16)
  per 512-wide q strip:
    for each 128-row k tile kt:
      S^T tile [k=128,q=512] = KT-chunk^T @ QT    (f32r, accum over D chunks)
      E = exp(S^T / sqrt(D)) -> bf16              (ACT, 2 half-tiles)
      per 128-col q subtile qt:
        O[qt]    += E[:,qt]^T @ V[kt]             (bf16 in, fp32 PSUM accum)
        dsum[qt] += E[:,qt]^T @ ones              (N=1 bf16 matmul)
    O[qt] *= 1/dsum[qt] (per-partition scalar, DVE/ACT alternating); DMA out

Inputs are declared float32r in DRAM (bit-identical to f32): no on-chip
casts. bf16 E/V keep the O-side LDWEIGHTS on the fast FWL path and make
the N=1 dsum matmuls legal (fp32r moving requires even free sizes).
DMA: xT strips + weights interleaved across both HW DGE queues
(strip-major so strip-s projections start as soon as strip s lands),
biases on the software DGE, outputs alternate sync/scalar.
"""

import numpy as np

import concourse.bass as bass
import concourse.tile as tile
from concourse import bacc, mybir
from concourse import bass_utils
from concourse.bass import ts
from contextlib import ExitStack

B, N, D = 8, 2048, 512
P = 128
NT = N // P      # 16 seq tiles
DC = D // P      # 4 d chunks
QS = 512         # q-strip width (one PSUM bank of fp32)
NS = N // QS     # 4 strips
QT_PER = QS // P # 4 q subtiles per strip
SOFTMAX_SCALE = 1.0 / float(np.sqrt(D))

F32 = mybir.dt.float32
F32R = mybir.dt.float32r
BF16 = mybir.dt.bfloat16
AF = mybir.ActivationFunctionType


def _build():
    nc = bacc.Bacc("TRN2", target_bir_lowering=False, debug=False)

    # xt is x^T [D, N]; weights are pre-rearranged [ki, ko*dout] (host-side)
    xt = nc.dram_tensor("xt", [D, N], F32R, kind="ExternalInput").ap()
    wq = nc.dram_tensor("wq", [P, DC * D], F32R, kind="ExternalInput").ap()
    bq = nc.dram_tensor("bq", [D], F32, kind="ExternalInput").ap()
    wk = nc.dram_tensor("wk", [P, DC * D], F32R, kind="ExternalInput").ap()
    bk = nc.dram_tensor("bk", [D], F32, kind="ExternalInput").ap()
    wv = nc.dram_tensor("wv", [P, DC * D], F32R, kind="ExternalInput").ap()
    bv = nc.dram_tensor("bv", [D], F32, kind="ExternalInput").ap()
    out = nc.dram_tensor("out", [N, D], BF16, kind="ExternalOutput").ap()

    with ExitStack() as ctx:
        tc = ctx.enter_context(tile.TileContext(nc))

        const = ctx.enter_context(tc.tile_pool(name="const", bufs=1))
        wpool = ctx.enter_context(tc.tile_pool(name="wpool", bufs=3))
        big = ctx.enter_context(tc.tile_pool(name="big", bufs=1))
        epool = ctx.enter_context(tc.tile_pool(name="epool", bufs=3))
        opool = ctx.enter_context(tc.tile_pool(name="opool", bufs=3))
        rpool = ctx.enter_context(tc.tile_pool(name="rpool", bufs=2))

        ones_col = const.tile([P, 1], BF16)
        nc.vector.memset(ones_col, 1.0)

        # biases via software DGE (tiny; keep HW queues free for x/weights)
        bq_sb = const.tile([P, DC], F32)
        nc.gpsimd.dma_start(bq_sb[:], bq.rearrange("(c p) -> p c", p=P))
        bk_sb = const.tile([P, DC], F32)
        nc.gpsimd.dma_start(bk_sb[:], bk.rearrange("(c p) -> p c", p=P))
        bv_rep = const.tile([P, D], F32)
        nc.gpsimd.dma_start(bv_rep[:], bv[None, :].to_broadcast((P, D)))

        # big persistent tensors
        xT = big.tile([P, DC, N], F32R)    # x^T: d on partitions
        QT = big.tile([P, DC, N], BF16)    # bf16: same PE rate, FWL LDW
        KT = big.tile([P, DC, N], BF16)
        V = big.tile([P, NT * D], BF16)    # natural: seq on partitions (bf16)

        # weights: wq/wk in co-major [ki, co, ko, 128] layout, loaded one
        # co-chunk (256KB) at a time interleaved with the xT strips so the
        # first projection matmul only waits for ~512KB of DMA; wv (needed
        # a strip later) rides along after strip 1
        wq_sb = wpool.tile([P, DC, DC, P], F32R, tag="wq")
        wk_sb = wpool.tile([P, DC, DC, P], F32R, tag="wk")
        wv_sb = wpool.tile([P, DC, D], F32R, tag="wv")
        w_sb = {"q": wq_sb, "k": wk_sb, "v": wv_sb}
        wq4 = wq.rearrange("p (co k) -> p co k", co=DC)
        wk4 = wk.rearrange("p (co k) -> p co k", co=DC)
        nc.sync.dma_start(w_sb["q"][:, 0], wq4[:, 0])
        nc.scalar.dma_start(w_sb["k"][:, 0], wk4[:, 0])

        # xT strips, strip-major, alternating HW queues so strip-s
        # projections can start as soon as strip s lands; remaining wq/wk
        # chunks follow strip 0, wv (only needed after all QT/KT) after
        # strip 1
        for s in range(NS):
            for c in range(DC):
                eng = nc.sync if (c % 2 == 0) else nc.scalar
                eng.dma_start(xT[:, c, ts(s, QS)], xt[ts(c, P), ts(s, QS)])
            if s == 0:
                for co in range(1, DC):
                    nc.sync.dma_start(w_sb["q"][:, co], wq4[:, co])
                    nc.scalar.dma_start(w_sb["k"][:, co], wk4[:, co])
            if s == 1:
                nc.sync.dma_start(w_sb["v"][:], wv)

        # ---- phase 1: projections, strip-major ----
        with tc.tile_pool(name="ps_proj", bufs=4, space="PSUM") as ps_proj:
            for s in range(NS):
                for co in range(DC):
                    for name, dst, b_sb in (("q", QT, bq_sb), ("k", KT, bk_sb)):
                        wr = w_sb[name]
                        pq = ps_proj.tile([P, QS], F32, tag="proj")
                        for ki in range(DC):
                            nc.tensor.matmul(
                                pq[:], wr[:, co, ki, :], xT[:, ki, ts(s, QS)],
                                start=(ki == 0), stop=(ki == DC - 1),
                            )
                        # bias add (per-partition) + round to bf16 on ACT
                        nc.scalar.activation(
                            dst[:, co, ts(s, QS)], pq[:], AF.Identity,
                            bias=b_sb[:, co:co + 1],
                        )
            # V last (phase 2 needs all of it anyway, and this lets wv's
            # DMA trail the wq/wk chunks): natural layout, bias along the
            # free dim via replicated tile
            wr = w_sb["v"]
            for m in range(NT):
                pv = ps_proj.tile([P, QS], F32, tag="proj")
                for ki in range(DC):
                    nc.tensor.matmul(
                        pv[:], xT[:, ki, ts(m, P)], wr[:, ki, :],
                        start=(ki == 0), stop=(ki == DC - 1),
                    )
                nc.vector.tensor_add(out=V[:, ts(m, D)], in0=pv[:],
                                     in1=bv_rep[:])

        # ---- phase 2: attention, natural-layout O accumulation ----
        with tc.tile_pool(name="ps_st", bufs=3, space="PSUM") as ps_st, \
             tc.tile_pool(name="ps_o", bufs=4, space="PSUM") as ps_o, \
             tc.tile_pool(name="ps_ds", bufs=1, space="PSUM") as ps_ds:
            for s in range(NS):
                o_ps = [ps_o.tile([P, QS], F32, tag="o", name=f"o_{s}_{qt}")
                        for qt in range(QT_PER)]
                dsum = ps_ds.tile([P, QT_PER], F32, tag="ds")
                # zero data; all dsum matmuls accumulate with start=False so
                # correctness doesn't depend on has_written clear granularity
                nc.vector.memset(dsum, 0.0)

                def s_block(kt):
                    st = ps_st.tile([P, QS], F32, tag="st")
                    for c in range(DC):
                        nc.tensor.matmul(
                            st[:], KT[:, c, ts(kt, P)], QT[:, c, ts(s, QS)],
                            start=(c == 0), stop=(c == DC - 1),
                        )
                    e = epool.tile([P, QS], BF16, tag="e")
                    nc.scalar.activation(e[:], st[:], AF.Exp,
                                         scale=SOFTMAX_SCALE)
                    return e

                # one-iteration software pipeline: the PE stream runs
                # S(kt+1) while ACT computes exp(kt), so O(kt) never waits
                e_cur = s_block(0)
                for kt in range(NT):
                    e_nxt = s_block(kt + 1) if kt + 1 < NT else None
                    for qt in range(QT_PER):
                        nc.tensor.matmul(
                            o_ps[qt][:], e_cur[:, ts(qt, P)], V[:, ts(kt, D)],
                            start=(kt == 0), stop=(kt == NT - 1),
                            skip_group_check=True,
                        )
                        nc.tensor.matmul(
                            dsum[:, qt:qt + 1], e_cur[:, ts(qt, P)],
                            ones_col[:],
                            start=False, stop=(kt == NT - 1),
                            skip_group_check=True,
                        )
                    e_cur = e_nxt
                r = rpool.tile([P, QT_PER], F32, tag="r")
                for qt in range(QT_PER):
                    # per-qt recip so the strip tail pipelines per tile
                    nc.vector.reciprocal(r[:, qt:qt + 1], dsum[:, qt:qt + 1])
                    ob = opool.tile([P, QS], BF16, tag="ob")
                    if qt % 2 == 0:
                        nc.vector.tensor_scalar_mul(ob[:], o_ps[qt][:],
                                                    r[:, qt:qt + 1])
                    else:
                        nc.scalar.activation(ob[:], o_ps[qt][:], AF.Identity,
                                             scale=r[:, qt:qt + 1])
                    eng = nc.sync if (qt % 2 == 0) else nc.scalar
                    eng.dma_start(out[ts(s * QT_PER + qt, P), :], ob[:])

    nc.compile()
    return nc


_CACHE = {}


def _get_nc():
    if "nc" not in _CACHE:
        _CACHE["nc"] = _build()
    return _CACHE["nc"]


def _host_prep_w_qk(w):
    # [din, dout] -> [ki, co, ko, 128] flattened, contiguous: one 2KB run
    # per partition per co so per-co weight DMAs use full descriptors
    w = np.asarray(w, dtype=np.float32)
    return np.ascontiguousarray(
        w.reshape(DC, P, DC, P).transpose(1, 2, 0, 3).reshape(P, DC * D))


def _host_prep_w_v(w):
    # [din, dout] -> [ki, ko, dout] flattened to [128, DC*D], contiguous
    w = np.asarray(w, dtype=np.float32)
    return np.ascontiguousarray(
        w.reshape(DC, P, D).transpose(1, 0, 2).reshape(P, DC * D))


def kernel(x, Wq_w, Wq_b, Wk_w, Wk_b, Wv_w, Wv_b, _trace=False, _tmpdir=None):
    nc = _get_nc()
    x = np.asarray(x, dtype=np.float32)
    xt = np.ascontiguousarray(x.transpose(0, 2, 1))   # [B, D, N]
    args = {
        "wq": _host_prep_w_qk(Wq_w),
        "bq": np.ascontiguousarray(Wq_b, np.float32),
        "wk": _host_prep_w_qk(Wk_w),
        "bk": np.ascontiguousarray(Wk_b, np.float32),
        "wv": _host_prep_w_v(Wv_w),
        "bv": np.ascontiguousarray(Wv_b, np.float32),
    }
    in_maps = [dict(args, xt=xt[i]) for i in range(B)]
    res = bass_utils.run_bass_kernel_spmd(
        nc, in_maps, core_ids=list(range(B)),
        trace=_trace, tmpdir=_tmpdir,
    )
    out = np.stack([np.asarray(r["out"], dtype=np.float32)
                    for r in res.results], axis=0)
    if _trace:
        kernel.last_results = res
    return out


if __name__ == "__main__":
    rng = np.random.default_rng(0)
    inputs = {
        "x": rng.standard_normal((B, N, D)).astype(np.float32),
        "Wq_w": (0.02 * rng.standard_normal((D, D))).astype(np.float32),
        "Wq_b": np.zeros(D, np.float32),
        "Wk_w": (0.02 * rng.standard_normal((D, D))).astype(np.float32),
        "Wk_b": np.zeros(D, np.float32),
        "Wv_w": (0.02 * rng.standard_normal((D, D))).astype(np.float32),
        "Wv_b": np.zeros(D, np.float32),
    }
    got = kernel(**inputs)
    print("out shape:", got.shape, got.dtype)
